# revision 9
# baseline (speedup 1.0000x reference)
"""Multi-head attention (B=2, S=2048, D=1024, H=16) on 8 trn2 NeuronCores.

Sharding: core c handles batch b = c // 4 and head-group g = c % 4
(4 heads = 256 hidden columns per core).  Each core computes its 4 heads'
attention plus the partial out-projection; the host sums the 4 partials
per batch and adds the (linear) bias terms.

All matmuls run in float32r (fp32 with 11-bit mantissa, full PE rate at
free-dim >= 256).  Inputs are pre-rounded to f32r on the host.

Layout per core (all DRAM tensors declared float32r unless noted):
  xqT, xkT, xvT : [1024, 2048]   x.T (host-transposed activations)
  wqT, wkT, wvT : [1024, 256]    W.T column slice for this head group
  woT           : [256, 1024]    Wo[:, J].T
  bq, bk        : [256]  fp32    bias slices (added via ACT during evac)
  outT (output) : [1024, 2048] fp32   partial (out @ Wo_J.T).T
"""
import os
import sys
import types

sys.path.insert(0, "/opt/trn_rl_repo")

import numpy as np


def _install_profshim():
    """Enable NTFF profiling under axon (KERNEL_TRACE=1 only)."""
    if "antenv.axon_hooks" in sys.modules:
        return
    try:
        from trn_agent_boot.trn_boot import _ntff_profile_via_ctypes

        hook = _ntff_profile_via_ctypes("/opt/axon/libaxon_pjrt.so")
        mod = types.ModuleType("antenv.axon_hooks")
        mod.get_axon_ntff_profile_hook = lambda: hook
        mod.set_axon_ntff_profile_hook = lambda h: None
        sys.modules["antenv.axon_hooks"] = mod
        import concourse.bass_utils as _bu

        _bu.upload_artifacts = lambda tmpdir: "local://unavailable"
    except Exception:
        pass

B = 2
S = 2048
D = 1024
H_PER_CORE = 4      # heads per core
DH = 64             # head dim
JG = 256            # hidden cols per core (4 heads * 64)
ND = D // 128       # 8 contraction d-tiles
NKT = S // 128      # 16 k-position tiles
NQC = 4             # q chunks
QC = S // NQC       # 512
SCALE = 1.0 / np.sqrt(DH)

_cache = {}


def round_fp32r(x: np.ndarray) -> np.ndarray:
    """Round fp32 to fp32r (11-bit mantissa, RNE, low 12 bits zero)."""
    x = np.ascontiguousarray(x, dtype=np.float32)
    u = x.view(np.uint32)
    r = (u + 0x7FF + ((u >> 12) & 1)) & np.uint32(0xFFFFF000)
    return r.view(np.float32)


def build_nc():
    import concourse.bacc as bacc
    import concourse.mybir as mybir
    import concourse.tile as tile

    f32 = mybir.dt.float32
    f32r = mybir.dt.float32r
    AF = mybir.ActivationFunctionType

    nc = bacc.Bacc("TRN2", target_bir_lowering=False)

    xqT = nc.dram_tensor("xqT", [D, S], f32r, kind="ExternalInput").ap()
    xkT = nc.dram_tensor("xkT", [D, S], f32r, kind="ExternalInput").ap()
    xvT = nc.dram_tensor("xvT", [D, S], f32r, kind="ExternalInput").ap()
    wqT = nc.dram_tensor("wqT", [D, JG], f32r, kind="ExternalInput").ap()
    wkT = nc.dram_tensor("wkT", [D, JG], f32r, kind="ExternalInput").ap()
    wvT = nc.dram_tensor("wvT", [D, JG], f32r, kind="ExternalInput").ap()
    woT = nc.dram_tensor("woT", [JG, D], f32r, kind="ExternalInput").ap()
    bq = nc.dram_tensor("bq", [JG], f32, kind="ExternalInput").ap()
    bk = nc.dram_tensor("bk", [JG], f32, kind="ExternalInput").ap()
    outT = nc.dram_tensor("outT", [D, S], f32, kind="ExternalOutput").ap()

    with tile.TileContext(nc) as tc:
        with (
            tc.tile_pool(name="xt", bufs=9) as xt_pool,
            tc.tile_pool(name="wts", bufs=1) as w_pool,
            tc.tile_pool(name="qkv", bufs=1) as qkv_pool,
            tc.tile_pool(name="attn", bufs=4) as attn_pool,
            tc.tile_pool(name="small", bufs=1) as small_pool,
            tc.tile_pool(name="nrm", bufs=3) as nrm_pool,
            tc.tile_pool(name="oev", bufs=3) as oev_pool,
        ):
            # ---- weight / bias loads (one DMA each) ----------------------
            wq_t = w_pool.tile([128, ND, JG], f32r, tag="wq")
            wk_t = w_pool.tile([128, ND, JG], f32r, tag="wk")
            wv_t = w_pool.tile([128, ND, JG], f32r, tag="wv")
            nc.sync.dma_start(wq_t[:], wqT.rearrange("(n p) j -> p n j", p=128))
            nc.sync.dma_start(wk_t[:], wkT.rearrange("(n p) j -> p n j", p=128))
            nc.sync.dma_start(wv_t[:], wvT.rearrange("(n p) j -> p n j", p=128))
            wo_t = w_pool.tile([128, 2, ND, 128], f32r, tag="wo")
            nc.sync.dma_start(
                wo_t[:], woT.rearrange("(a p) (n m) -> p a n m", p=128, m=128)
            )
            bq_t = small_pool.tile([128, 2], f32, tag="bq")
            bk_t = small_pool.tile([128, 2], f32, tag="bk")
            nc.sync.dma_start(bq_t[:], bq.rearrange("(a p) -> p a", p=128))
            nc.sync.dma_start(bk_t[:], bk.rearrange("(a p) -> p a", p=128))
            ones1 = small_pool.tile([1, DH], f32, tag="ones1")
            nc.vector.memset(ones1[:], 1.0)

            # ---- persistent activation tensors ---------------------------
            # Q_T / K_T: [256, 2048] as 2 m-tiles of [128, 2048]
            q_t = [qkv_pool.tile([128, S], f32r, tag=f"qt{m}", name=f"qt{m}")
                   for m in range(2)]
            k_t = [qkv_pool.tile([128, S], f32r, tag=f"kt{m}", name=f"kt{m}")
                   for m in range(2)]
            # V (natural layout) + ones column per head: 16 s-tiles
            # [128, 4*65]; view per head h: [:, h, 0:64] = Vh, [:, h, 64] = 1
            v_t = [qkv_pool.tile([128, H_PER_CORE, DH + 1], f32r,
                                 tag=f"v{s}", name=f"v{s}")
                   for s in range(NKT)]
            # attention output (transposed): 2 m-tiles [128, 2048]
            ao_t = [qkv_pool.tile([128, S], f32r, tag=f"ao{m}", name=f"ao{m}")
                    for m in range(2)]

            # ---- phase 1: projections (d-outer, PSUM-resident) -----------
            with tc.tile_pool(name="proj_psum", bufs=1, space="PSUM") as pp:
                # Q_T then K_T: out [m*128 + p, s]; 2 m-tiles x 4 s-chunks
                for name, w_full, x_dram, dst, bias in (
                    ("q", wq_t, xqT, q_t, bq_t),
                    ("k", wk_t, xkT, k_t, bk_t),
                ):
                    xs = []
                    for d in range(ND):
                        xd = xt_pool.tile([128, S], f32r, tag="xT",
                                          name=f"x{name}{d}")
                        nc.sync.dma_start(
                            xd[:], x_dram[d * 128:(d + 1) * 128, :]
                        )
                        xs.append(xd)
                    ps = {
                        (m, c): pp.tile([128, QC], f32, tag=f"pp{m}{c}",
                                        name=f"ps{name}{m}{c}")
                        for m in range(2) for c in range(NQC)
                    }
                    for d in range(ND):
                        for m in range(2):
                            for c in range(NQC):
                                nc.tensor.matmul(
                                    ps[(m, c)][:],
                                    w_full[:, d, m * 128:(m + 1) * 128],
                                    xs[d][:, c * QC:(c + 1) * QC],
                                    start=(d == 0),
                                    stop=(d == ND - 1),
                                )
                    for m in range(2):
                        for c in range(NQC):
                            nc.scalar.activation(
                                dst[m][:, c * QC:(c + 1) * QC],
                                ps[(m, c)][:],
                                AF.Identity,
                                bias=bias[:, m:m + 1],
                            )

                # V: natural layout, two halves of 8 s-tiles
                xs = []
                for d in range(ND):
                    xd = xt_pool.tile([128, S], f32r, tag="xT", name=f"xv{d}")
                    nc.sync.dma_start(xd[:], xvT[d * 128:(d + 1) * 128, :])
                    xs.append(xd)
                for half in range(2):
                    ps = {
                        s: pp.tile([128, JG], f32,
                                   tag=f"pp{s % 2}{s // 2 % 4}",
                                   name=f"psv{s}")
                        for s in range(half * 8, half * 8 + 8)
                    }
                    for d in range(ND):
                        for s in range(half * 8, half * 8 + 8):
                            nc.tensor.matmul(
                                ps[s][:],
                                xs[d][:, s * 128:(s + 1) * 128],
                                wv_t[:, d, :],
                                start=(d == 0),
                                stop=(d == ND - 1),
                            )
                    for s in range(half * 8, half * 8 + 8):
                        # copy into the 4 per-head 64-col slots
                        nc.scalar.activation(
                            v_t[s][:, :, 0:DH],
                            ps[s][:].rearrange("p (h d) -> p h d", d=DH),
                            AF.Copy,
                        )
                ones4 = small_pool.tile([128, H_PER_CORE], f32, tag="ones4")
                nc.vector.memset(ones4[:], 1.0)
                for s in range(NKT):
                    nc.scalar.activation(
                        v_t[s][:, :, DH], ones4[:], AF.Copy
                    )

            # ---- phase 2+3: attention + out-projection -------------------
            with tc.tile_pool(name="apsum", bufs=1, space="PSUM") as ap_pool:
                for qc in range(NQC):
                    qsl = slice(qc * QC, (qc + 1) * QC)
                    for hp in range(2):          # head pairs (2h, 2h+1)
                        av_ps = {}
                        for hh in range(2):      # parity within pair
                            h = hp * 2 + hh
                            po = hh * DH         # partition offset 0 / 64
                            av_ps[hh] = ap_pool.tile(
                                [DH + 1, QC], f32, tag=f"av{hh}",
                                name=f"av{hh}_{hp}_{qc}",
                            )
                            for kt in range(NKT):
                                sc_ps = ap_pool.tile(
                                    [128, QC], f32, tag=f"sc{kt % 3}"
                                )
                                nc.tensor.matmul(
                                    sc_ps[:],
                                    k_t[hp][po:po + DH,
                                            kt * 128:(kt + 1) * 128],
                                    q_t[hp][po:po + DH, qsl],
                                    start=True, stop=True,
                                )
                                at = attn_pool.tile([128, QC], f32r, tag="at")
                                nc.scalar.activation(
                                    at[:], sc_ps[:], AF.Exp, scale=float(SCALE)
                                )
                                nc.tensor.matmul(
                                    av_ps[hh][:],
                                    v_t[kt][:, h, :],
                                    at[:],
                                    start=(kt == 0),
                                    stop=(kt == NKT - 1),
                                )
                        # normalize both heads of the pair
                        for hh in range(2):
                            h = hp * 2 + hh
                            po = hh * DH
                            rc = nrm_pool.tile([1, QC], f32, tag="rc")
                            nc.vector.reciprocal(rc[:], av_ps[hh][DH:DH + 1, :])
                            bc_ps = ap_pool.tile([DH, QC], f32, tag="bc")
                            nc.tensor.matmul(
                                bc_ps[:], ones1[:], rc[:],
                                start=True, stop=True,
                            )
                            rb = nrm_pool.tile([DH, QC], f32, tag="rb")
                            nc.vector.tensor_copy(rb[:], bc_ps[:])
                            nc.vector.tensor_mul(
                                ao_t[hp][po:po + DH, qsl],
                                av_ps[hh][0:DH, :],
                                rb[:],
                            )
                    # out-projection for this q chunk
                    for im in range(ND):
                        wo_ps = ap_pool.tile([128, QC], f32, tag="wo")
                        for jk in range(2):
                            nc.tensor.matmul(
                                wo_ps[:],
                                wo_t[:, jk, im, :],
                                ao_t[jk][:, qsl],
                                start=(jk == 0),
                                stop=(jk == 1),
                            )
                        ot = oev_pool.tile([128, QC], f32, tag="ot")
                        nc.vector.tensor_copy(ot[:], wo_ps[:])
                        nc.sync.dma_start(
                            outT[im * 128:(im + 1) * 128, qsl], ot[:]
                        )

    nc.compile()
    return nc


def _get_nc():
    if "nc" not in _cache:
        _cache["nc"] = build_nc()
    return _cache["nc"]


def kernel(q, k, v, Wq, bq, Wk, bk, Wv, bv, Wo, bo, **_unused):
    from concourse.bass_utils import run_bass_kernel_spmd

    q = np.asarray(q, dtype=np.float32)
    k = np.asarray(k, dtype=np.float32)
    v = np.asarray(v, dtype=np.float32)
    Wq = np.asarray(Wq, dtype=np.float32)
    Wk = np.asarray(Wk, dtype=np.float32)
    Wv = np.asarray(Wv, dtype=np.float32)
    Wo = np.asarray(Wo, dtype=np.float32)
    bq = np.asarray(bq, dtype=np.float32)
    bk = np.asarray(bk, dtype=np.float32)
    bv = np.asarray(bv, dtype=np.float32)
    bo = np.asarray(bo, dtype=np.float32)

    nc = _get_nc()

    xT = {b: {} for b in range(B)}
    for b in range(B):
        xT[b]["q"] = round_fp32r(q[b].T)
        xT[b]["k"] = round_fp32r(k[b].T)
        xT[b]["v"] = round_fp32r(v[b].T)

    wslices = []
    for g in range(4):
        J = slice(g * JG, (g + 1) * JG)
        wslices.append({
            "wqT": round_fp32r(Wq.T[:, J]),
            "wkT": round_fp32r(Wk.T[:, J]),
            "wvT": round_fp32r(Wv.T[:, J]),
            "woT": round_fp32r(Wo[:, J].T),
            "bq": np.ascontiguousarray(bq[J]),
            "bk": np.ascontiguousarray(bk[J]),
        })

    in_maps = []
    for c in range(8):
        b, g = c // 4, c % 4
        m = {
            "xqT": xT[b]["q"], "xkT": xT[b]["k"], "xvT": xT[b]["v"],
        }
        m.update(wslices[g])
        in_maps.append(m)

    trace = bool(int(os.environ.get("KERNEL_TRACE", "0")))
    if trace:
        _install_profshim()
    res = run_bass_kernel_spmd(
        nc, in_maps, core_ids=list(range(8)), trace=trace
    )
    _cache["exec_time_ns"] = res.exec_time_ns
    parts = [r["outT"] for r in res.results]

    # host reduce: sum the 4 head-group partials per batch, transpose,
    # add the linear bias terms (bo + Wo @ bv, exact fold)
    const_row = bo + Wo @ bv
    out = np.empty((B, S, D), dtype=np.float32)
    for b in range(B):
        acc = parts[4 * b].copy()
        for g in range(1, 4):
            acc += parts[4 * b + g]
        out[b] = acc.T + const_row
    return out


# revision 11
# speedup vs baseline: 1.3061x; 1.3061x over previous
"""Multi-head attention (B=2, S=2048, D=1024, H=16) on 8 trn2 NeuronCores.

Sharding: core c handles batch b = c // 4 and head-group g = c % 4
(4 heads = 256 hidden columns per core).  Each core computes its 4 heads'
attention plus the partial out-projection; the host sums the 4 partials
per batch and adds the (linear) bias terms (bo + Wo @ bv) exactly.

v2: all matmuls in bf16 (1 cycle/row on the PE; fp32 accumulation in
PSUM), exp in [128,1024] tiles, reciprocal_approx_fast for softmax
denominators.

Layout per core (DRAM tensors bf16 unless noted):
  xqT, xkT, xvT : [1024, 2048]   x.T (host-transposed activations)
  wqT, wkT, wvT : [1024, 256]    W.T column slice for this head group
  woT           : [256, 1024]    Wo[:, J].T
  bq, bk        : [256]  fp32    bias slices (added via ACT during evac)
  outT (output) : [1024, 2048] fp32   partial (out @ Wo_J.T).T
"""
import os
import sys
import types

sys.path.insert(0, "/opt/trn_rl_repo")

import numpy as np

B = 2
S = 2048
D = 1024
H_PER_CORE = 4      # heads per core
DH = 64             # head dim
JG = 256            # hidden cols per core (4 heads * 64)
ND = D // 128       # 8 contraction d-tiles
NKT = S // 128      # 16 k-position tiles
QC = 512
PC = 1024           # processed q columns per pass (2 chunks of 512)
SCALE = 1.0 / np.sqrt(DH)

_cache = {}


def _install_profshim():
    """Enable NTFF profiling under axon (KERNEL_TRACE=1 only)."""
    if "antenv.axon_hooks" in sys.modules:
        return
    try:
        from trn_agent_boot.trn_boot import _ntff_profile_via_ctypes

        hook = _ntff_profile_via_ctypes("/opt/axon/libaxon_pjrt.so")
        mod = types.ModuleType("antenv.axon_hooks")
        mod.get_axon_ntff_profile_hook = lambda: hook
        mod.set_axon_ntff_profile_hook = lambda h: None
        sys.modules["antenv.axon_hooks"] = mod
        import concourse.bass_utils as _bu

        _bu.upload_artifacts = lambda tmpdir: "local://unavailable"
    except Exception:
        pass


def build_nc():
    import concourse.bacc as bacc
    import concourse.mybir as mybir
    import concourse.tile as tile

    f32 = mybir.dt.float32
    bf16 = mybir.dt.bfloat16
    AF = mybir.ActivationFunctionType

    nc = bacc.Bacc("TRN2", target_bir_lowering=False)

    xqT = nc.dram_tensor("xqT", [D, S], bf16, kind="ExternalInput").ap()
    xkT = nc.dram_tensor("xkT", [D, S], bf16, kind="ExternalInput").ap()
    xvT = nc.dram_tensor("xvT", [D, S], bf16, kind="ExternalInput").ap()
    wqT = nc.dram_tensor("wqT", [D, JG], bf16, kind="ExternalInput").ap()
    wkT = nc.dram_tensor("wkT", [D, JG], bf16, kind="ExternalInput").ap()
    wvT = nc.dram_tensor("wvT", [D, JG], bf16, kind="ExternalInput").ap()
    woT = nc.dram_tensor("woT", [JG, D], bf16, kind="ExternalInput").ap()
    bq = nc.dram_tensor("bq", [JG], f32, kind="ExternalInput").ap()
    bk = nc.dram_tensor("bk", [JG], f32, kind="ExternalInput").ap()
    outT = nc.dram_tensor("outT", [D, S], f32, kind="ExternalOutput").ap()

    with tile.TileContext(nc) as tc:
        with (
            tc.tile_pool(name="xt", bufs=16) as xt_pool,
            tc.tile_pool(name="wts", bufs=1) as w_pool,
            tc.tile_pool(name="qkv", bufs=1) as qkv_pool,
            tc.tile_pool(name="attn", bufs=3) as attn_pool,
            tc.tile_pool(name="small", bufs=1) as small_pool,
            tc.tile_pool(name="nrm", bufs=2) as nrm_pool,
            tc.tile_pool(name="oev", bufs=3) as oev_pool,
        ):
            # ---- weight / bias loads (one DMA each) ----------------------
            wq_t = w_pool.tile([128, ND, JG], bf16, tag="wq")
            wk_t = w_pool.tile([128, ND, JG], bf16, tag="wk")
            wv_t = w_pool.tile([128, ND, JG], bf16, tag="wv")
            nc.sync.dma_start(wq_t[:], wqT.rearrange("(n p) j -> p n j", p=128))
            nc.sync.dma_start(wk_t[:], wkT.rearrange("(n p) j -> p n j", p=128))
            nc.sync.dma_start(wv_t[:], wvT.rearrange("(n p) j -> p n j", p=128))
            wo_t = w_pool.tile([128, 2, ND, 128], bf16, tag="wo")
            nc.sync.dma_start(
                wo_t[:], woT.rearrange("(a p) (n m) -> p a n m", p=128, m=128)
            )
            bq_t = small_pool.tile([128, 2], f32, tag="bq")
            bk_t = small_pool.tile([128, 2], f32, tag="bk")
            nc.sync.dma_start(bq_t[:], bq.rearrange("(a p) -> p a", p=128))
            nc.sync.dma_start(bk_t[:], bk.rearrange("(a p) -> p a", p=128))
            ones1 = small_pool.tile([1, DH], f32, tag="ones1")
            nc.vector.memset(ones1[:], 1.0)

            # ---- persistent activation tensors ---------------------------
            q_t = [qkv_pool.tile([128, S], bf16, tag=f"qt{m}", name=f"qt{m}")
                   for m in range(2)]
            k_t = [qkv_pool.tile([128, S], bf16, tag=f"kt{m}", name=f"kt{m}")
                   for m in range(2)]
            # V (natural layout) + ones column per head: 16 s-tiles
            v_t = [qkv_pool.tile([128, H_PER_CORE, DH + 1], bf16,
                                 tag=f"v{s}", name=f"v{s}")
                   for s in range(NKT)]
            ao_t = [qkv_pool.tile([128, S], bf16, tag=f"ao{m}", name=f"ao{m}")
                    for m in range(2)]

            # ---- phase 1: projections (d-outer, PSUM-resident) -----------
            with tc.tile_pool(name="proj_psum", bufs=1, space="PSUM") as pp:
                for name, w_full, x_dram, dst, bias in (
                    ("q", wq_t, xqT, q_t, bq_t),
                    ("k", wk_t, xkT, k_t, bk_t),
                ):
                    xs = []
                    for d in range(ND):
                        xd = xt_pool.tile([128, S], bf16, tag="xT",
                                          name=f"x{name}{d}")
                        nc.sync.dma_start(
                            xd[:], x_dram[d * 128:(d + 1) * 128, :]
                        )
                        xs.append(xd)
                    ps = {
                        (m, c): pp.tile([128, QC], f32, tag=f"pp{m}{c}",
                                        name=f"ps{name}{m}{c}")
                        for m in range(2) for c in range(4)
                    }
                    for d in range(ND):
                        for m in range(2):
                            for c in range(4):
                                nc.tensor.matmul(
                                    ps[(m, c)][:],
                                    w_full[:, d, m * 128:(m + 1) * 128],
                                    xs[d][:, c * QC:(c + 1) * QC],
                                    start=(d == 0),
                                    stop=(d == ND - 1),
                                )
                    for m in range(2):
                        for c in range(4):
                            nc.scalar.activation(
                                dst[m][:, c * QC:(c + 1) * QC],
                                ps[(m, c)][:],
                                AF.Identity,
                                bias=bias[:, m:m + 1],
                            )

                # V: natural layout, two halves of 8 s-tiles
                xs = []
                for d in range(ND):
                    xd = xt_pool.tile([128, S], bf16, tag="xT", name=f"xv{d}")
                    nc.sync.dma_start(xd[:], xvT[d * 128:(d + 1) * 128, :])
                    xs.append(xd)
                for half in range(2):
                    ps = {
                        s: pp.tile([128, JG], f32,
                                   tag=f"pp{s % 2}{s // 2 % 4}",
                                   name=f"psv{s}")
                        for s in range(half * 8, half * 8 + 8)
                    }
                    for d in range(ND):
                        for s in range(half * 8, half * 8 + 8):
                            nc.tensor.matmul(
                                ps[s][:],
                                xs[d][:, s * 128:(s + 1) * 128],
                                wv_t[:, d, :],
                                start=(d == 0),
                                stop=(d == ND - 1),
                            )
                    for s in range(half * 8, half * 8 + 8):
                        nc.scalar.activation(
                            v_t[s][:, :, 0:DH],
                            ps[s][:].rearrange("p (h d) -> p h d", d=DH),
                            AF.Copy,
                        )
                ones4 = small_pool.tile([128, H_PER_CORE], f32, tag="ones4")
                nc.vector.memset(ones4[:], 1.0)
                for s in range(NKT):
                    nc.scalar.activation(
                        v_t[s][:, :, DH], ones4[:], AF.Copy
                    )

            # ---- phase 2+3: attention + out-projection -------------------
            # per q-column pair p (1024 cols), head pair hp, head parity hh
            with tc.tile_pool(name="apsum", bufs=1, space="PSUM") as ap_pool:
                for p in range(2):
                    pc0 = p * PC
                    psl = slice(pc0, pc0 + PC)
                    for hp in range(2):
                        av_ps = {}
                        for hh in range(2):
                            h = hp * 2 + hh
                            po = hh * DH
                            av_ps[hh] = ap_pool.tile(
                                [DH + 1, PC], f32, tag=f"av{hh}",
                                name=f"av{hh}_{hp}_{p}",
                            )
                            for kt in range(NKT):
                                sc_ps = ap_pool.tile(
                                    [128, PC], f32, tag=f"sc{kt % 2}",
                                    name=f"sc_{hh}_{kt}",
                                )
                                for n in range(2):
                                    nc.tensor.matmul(
                                        sc_ps[:, n * QC:(n + 1) * QC],
                                        k_t[hp][po:po + DH,
                                                kt * 128:(kt + 1) * 128],
                                        q_t[hp][po:po + DH,
                                                pc0 + n * QC:
                                                pc0 + (n + 1) * QC],
                                        start=True, stop=True,
                                    )
                                at = attn_pool.tile([128, PC], bf16, tag="at")
                                nc.scalar.activation(
                                    at[:], sc_ps[:], AF.Exp, scale=float(SCALE)
                                )
                                for n in range(2):
                                    nc.tensor.matmul(
                                        av_ps[hh][:, n * QC:(n + 1) * QC],
                                        v_t[kt][:, h, :],
                                        at[:, n * QC:(n + 1) * QC],
                                        start=(kt == 0),
                                        stop=(kt == NKT - 1),
                                    )
                        # normalize both heads of the pair
                        for hh in range(2):
                            po = hh * DH
                            rc = nrm_pool.tile([1, PC], f32, tag="rc")
                            nc.vector.reciprocal(
                                rc[:], av_ps[hh][DH:DH + 1, :]
                            )
                            bc_ps = ap_pool.tile(
                                [DH, PC], f32, tag=f"sc{hh}",
                                name=f"bc{hh}_{hp}_{p}",
                            )
                            for n in range(2):
                                nc.tensor.matmul(
                                    bc_ps[:, n * QC:(n + 1) * QC],
                                    ones1[:],
                                    rc[:, n * QC:(n + 1) * QC],
                                    start=True, stop=True,
                                )
                            rb = nrm_pool.tile([DH, PC], f32, tag="rb")
                            nc.vector.tensor_copy(rb[:], bc_ps[:])
                            nc.vector.tensor_mul(
                                ao_t[hp][po:po + DH, psl],
                                av_ps[hh][0:DH, :],
                                rb[:],
                            )
                    # out-projection for this column pair
                    for im in range(ND):
                        for n in range(2):
                            wo_ps = ap_pool.tile(
                                [128, QC], f32, tag=f"sc{n}",
                                name=f"wo{im}_{n}_{p}",
                            )
                            for jk in range(2):
                                nc.tensor.matmul(
                                    wo_ps[:],
                                    wo_t[:, jk, im, :],
                                    ao_t[jk][:, pc0 + n * QC:
                                             pc0 + (n + 1) * QC],
                                    start=(jk == 0),
                                    stop=(jk == 1),
                                )
                            ot = oev_pool.tile([128, QC], f32, tag="ot")
                            nc.vector.tensor_copy(ot[:], wo_ps[:])
                            nc.sync.dma_start(
                                outT[im * 128:(im + 1) * 128,
                                     pc0 + n * QC:pc0 + (n + 1) * QC],
                                ot[:],
                            )

    nc.compile()
    return nc


def _get_nc():
    if "nc" not in _cache:
        _cache["nc"] = build_nc()
    return _cache["nc"]


def kernel(q, k, v, Wq, bq, Wk, bk, Wv, bv, Wo, bo, **_unused):
    import ml_dtypes
    from concourse.bass_utils import run_bass_kernel_spmd

    bf = ml_dtypes.bfloat16
    q = np.asarray(q, dtype=np.float32)
    k = np.asarray(k, dtype=np.float32)
    v = np.asarray(v, dtype=np.float32)
    Wq = np.asarray(Wq, dtype=np.float32)
    Wk = np.asarray(Wk, dtype=np.float32)
    Wv = np.asarray(Wv, dtype=np.float32)
    Wo = np.asarray(Wo, dtype=np.float32)
    bq = np.asarray(bq, dtype=np.float32)
    bk = np.asarray(bk, dtype=np.float32)
    bv = np.asarray(bv, dtype=np.float32)
    bo = np.asarray(bo, dtype=np.float32)

    nc = _get_nc()

    xT = {b: {} for b in range(B)}
    for b in range(B):
        xT[b]["q"] = np.ascontiguousarray(q[b].T).astype(bf)
        xT[b]["k"] = np.ascontiguousarray(k[b].T).astype(bf)
        xT[b]["v"] = np.ascontiguousarray(v[b].T).astype(bf)

    wslices = []
    for g in range(4):
        J = slice(g * JG, (g + 1) * JG)
        wslices.append({
            "wqT": np.ascontiguousarray(Wq.T[:, J]).astype(bf),
            "wkT": np.ascontiguousarray(Wk.T[:, J]).astype(bf),
            "wvT": np.ascontiguousarray(Wv.T[:, J]).astype(bf),
            "woT": np.ascontiguousarray(Wo[:, J].T).astype(bf),
            "bq": np.ascontiguousarray(bq[J]),
            "bk": np.ascontiguousarray(bk[J]),
        })

    in_maps = []
    for c in range(8):
        b, g = c // 4, c % 4
        m = {
            "xqT": xT[b]["q"], "xkT": xT[b]["k"], "xvT": xT[b]["v"],
        }
        m.update(wslices[g])
        in_maps.append(m)

    trace = bool(int(os.environ.get("KERNEL_TRACE", "0")))
    if trace:
        _install_profshim()
    res = run_bass_kernel_spmd(
        nc, in_maps, core_ids=list(range(8)), trace=trace
    )
    _cache["exec_time_ns"] = res.exec_time_ns
    parts = [r["outT"] for r in res.results]

    # host reduce: sum the 4 head-group partials per batch, transpose,
    # add the linear bias terms (bo + Wo @ bv, exact fold)
    const_row = bo + Wo @ bv
    out = np.empty((B, S, D), dtype=np.float32)
    for b in range(B):
        acc = parts[4 * b].copy()
        for g in range(1, 4):
            acc += parts[4 * b + g]
        out[b] = acc.T + const_row
    return out
